# revision 9
# baseline (speedup 1.0000x reference)
"""GAT layer (edge softmax + weighted scatter) on 8 Trainium2 NeuronCores.

Strategy (dst-range sharding, no collectives):
  - Nodes are split into 8 contiguous dst ranges of 6250; since `dst` is
    sorted, each core owns a contiguous edge range and all of its
    destination segments -> no cross-core reduction.
  - Per core, edges are split into two streams by src < 32768 (so gather
    indices fit int16 for dma_gather), then greedily packed into chunks of
    <=128 edges covering <=32 consecutive dst nodes (whole segments only).
  - Per chunk (one 128-slot column): dma_gather pulls h[src] rows; the
    edge score e = (z.w1) + a_dst[dst] is built with a DVE dot (z.w1,
    folded into the gathered rows' w1-scaling) plus a one-hot matmul for
    the a_dst broadcast; p = exp(leaky_relu(e)) without max-subtraction
    (scores are O(1), exp cannot overflow); a one-hot matmul accumulates
    p-weighted rows and the softmax denominator into PSUM; results are
    scatter-added into a per-node accumulator table in DRAM.
  - A final pass divides by the denominator (and undoes the w1 fold).

The same program runs SPMD on all 8 cores; all data-dependent structure
(chunk windows, gather/scatter indices) is carried as per-core int16/f32
index arrays built on the host from src/dst only.
"""
import sys

sys.path.insert(0, "/opt/trn_rl_repo")

import numpy as np

N, F, E, NCORES = 50000, 64, 800000, 8
NLOC = N // NCORES            # 6250 nodes per core
K = 32                        # chunks per super-step
WMAX = 32                     # max window (dst span) per chunk
HALF = 32768                  # int16 split of the gather table
TCOLS = 49                    # 128*49 = 6272 >= NLOC
NPAD = 128 * TCOLS            # padded local node count
TA_ROWS = NPAD + 1            # a_dst table rows (+1 dump row)
DUMP = NPAD                   # dump row id (6272)
NEG_SLOPE = 0.01


# ---------------------------------------------------------------- host prep
def _pack_stream(counts):
    """Greedy chunking: whole segments, <=128 edges, window <=WMAX nodes.
    Returns list of (node_start, span, edge_start, n_edges)."""
    chunks = []
    node = 0
    epos = 0
    n = len(counts)
    while node < n:
        ns = node
        ec = 0
        while node < n and node - ns < WMAX:
            c = int(counts[node])
            if ec + c > 128:
                break
            ec += c
            node += 1
        assert node > ns, f"single segment with {counts[ns]} > 128 edges"
        chunks.append((ns, node - ns, epos, ec))
        epos += ec
    return chunks


def _wrap16(flat):
    """dma_gather/scatter idx layout: idx k at (partition k%16, col k//16),
    replicated across the 8 q7 cores (partition groups of 16)."""
    a = flat.reshape(-1, 16).T
    return np.ascontiguousarray(np.tile(a, (8, 1)), dtype=np.int16)


def _prep_core(src_loc, dst_loc):
    """Per-core, per-stream chunk structures."""
    streams = []
    for is_hi in (0, 1):
        m = (src_loc >= HALF) if is_hi else (src_loc < HALF)
        s_src = src_loc[m]
        s_dst = dst_loc[m]
        counts = np.bincount(s_dst, minlength=NLOC)
        chunks = _pack_stream(counts)
        streams.append((s_src, s_dst, chunks))
    return streams


def _build_arrays(streams, s_lo, s_hi):
    """Build the per-core index arrays, padded to uniform super counts."""
    S = s_lo + s_hi
    idxg = np.zeros((S, 4096), np.int64)          # gather idx per slot k
    drel = np.full((S, 128, K), -1.0, np.float32)  # window-relative dst
    awi = np.full((S, 128, 8), DUMP, np.int64)     # a_dst-table row per (p,g)
    sci = np.full((S, 1024), DUMP, np.int64)       # acc row per idx position i
    for is_hi, (s_src, s_dst, chunks) in enumerate(streams):
        s0 = 0 if not is_hi else s_lo
        n_sup = s_lo if not is_hi else s_hi
        assert len(chunks) <= n_sup * K
        for ci, (ns, span, es, ec) in enumerate(chunks):
            s = s0 + ci // K
            c = ci % K
            # gather: slot k = c*128 + p
            vals = s_src[es : es + ec].astype(np.int64)
            if is_hi:
                vals = vals - HALF
            idxg[s, c * 128 : c * 128 + ec] = vals
            # dst_rel
            drel[s, :ec, c] = s_dst[es : es + ec] - ns
            # a-window gather: slot (p = 32j+w, g) for chunk c = 4g+j
            j, g = c % 4, c // 4
            w = np.arange(span)
            awi[s, 32 * j + w, g] = ns + w
            # scatter: src slot (p = 32j+w, g) of the relayouted
            # [128, 8, 65] tile; idx position i = g*128 + p
            sci[s, g * 128 + 32 * j + w] = ns + w
    return idxg, drel, awi, sci


# ------------------------------------------------------------- bass program
def _build_program(s_lo, s_hi):
    import os
    STAGE = int(os.environ.get("GAT_STAGE", "9"))
    import concourse.bacc as bacc
    import concourse.tile as tile
    import concourse.mybir as mybir
    from concourse import bass
    from concourse.masks import make_identity

    f32, i16 = mybir.dt.float32, mybir.dt.int16
    AF = mybir.ActivationFunctionType
    OP = mybir.AluOpType
    S = s_lo + s_hi

    nc = bacc.Bacc("TRN2", target_bir_lowering=False, debug=False,
                   num_devices=NCORES)
    h_t = nc.dram_tensor("h", [N, F], f32, kind="ExternalInput")
    hs_t = nc.dram_tensor("h_slice", [NPAD, F], f32, kind="ExternalInput")
    w_t = nc.dram_tensor("attn_w", [2 * F], f32, kind="ExternalInput")
    idxg_t = nc.dram_tensor("idxg", [S, 128, 256], i16, kind="ExternalInput")
    drel_t = nc.dram_tensor("drel", [S, 128, K], f32, kind="ExternalInput")
    awi_t = nc.dram_tensor("awi", [S, 128, 64], i16, kind="ExternalInput")
    sci_t = nc.dram_tensor("sci", [S, 128, 64], i16, kind="ExternalInput")
    acc_t = nc.dram_tensor("acc", [TA_ROWS, 128], f32, kind="ExternalOutput")
    out_t = nc.dram_tensor("out", [NPAD, F], f32, kind="ExternalOutput")
    ta_t = nc.dram_tensor("ta", [TA_ROWS, F], f32, kind="Internal")

    def bc_ap(tensor, offset, ap):
        return bass.AP(tensor=tensor, offset=offset, ap=ap)

    with tile.TileContext(nc) as tc:
        with tc.tile_pool(name="const", bufs=1) as const, \
             tc.tile_pool(name="pre", bufs=1) as pre, \
             tc.tile_pool(name="ldi", bufs=3) as ldi, \
             tc.tile_pool(name="big", bufs=3) as big, \
             tc.tile_pool(name="med", bufs=3) as med, \
             tc.tile_pool(name="tiny", bufs=4) as tiny, \
             tc.tile_pool(name="orows", bufs=3) as orows, \
             tc.tile_pool(name="ps_st", bufs=1, space="PSUM") as ps_st, \
             tc.tile_pool(name="ps_d", bufs=2, space="PSUM") as ps_d, \
             tc.tile_pool(name="ps_o", bufs=3, space="PSUM") as ps_o:

            # ---------------- constants
            ident = const.tile([128, 128], f32)
            make_identity(nc, ident[:])
            w1t = const.tile([128, F], f32)
            nc.gpsimd.dma_start(out=w1t[:], in_=bc_ap(w_t, 0, [[0, 128], [1, F]]))
            w2t = const.tile([128, F], f32)
            nc.gpsimd.dma_start(out=w2t[:], in_=bc_ap(w_t, F, [[0, 128], [1, F]]))
            iota32 = const.tile([128, WMAX], f32)
            nc.gpsimd.iota(iota32[:], pattern=[[1, WMAX]], base=0,
                           channel_multiplier=0,
                           allow_small_or_imprecise_dtypes=True)
            jmask = const.tile([128, 4], f32)
            nc.vector.memset(jmask[:], 0.0)
            for j in range(4):
                nc.vector.memset(jmask[32 * j : 32 * j + 32, j : j + 1], 1.0)
            rw1 = const.tile([128, F], f32)
            nc.vector.reciprocal(rw1[:], w1t[:])

            # ---------------- preamble: a_dst table
            hs = pre.tile([128, TCOLS, F], f32)
            nc.sync.dma_start(out=hs[:],
                              in_=hs_t[:].rearrange("(p t) f -> p t f", p=128))
            hw2 = pre.tile([128, TCOLS, F], f32)
            nc.vector.tensor_tensor(
                out=hw2[:], in0=hs[:],
                in1=w2t[:, None, :].to_broadcast([128, TCOLS, F]), op=OP.mult)
            a_sb = pre.tile([128, TCOLS], f32)
            nc.vector.tensor_reduce(out=a_sb[:], in_=hw2[:],
                                    axis=mybir.AxisListType.X, op=OP.add)
            a64 = pre.tile([128, TCOLS, F], f32)
            nc.vector.tensor_copy(
                out=a64[:],
                in_=a_sb[:, :, None].to_broadcast([128, TCOLS, F]))
            nc.sync.dma_start(
                out=ta_t[0:NPAD, :].rearrange("(p t) f -> p t f", p=128),
                in_=a64[:])
            zrow = pre.tile([1, F], f32)
            nc.vector.memset(zrow[:], 0.0)
            nc.sync.dma_start(out=ta_t[NPAD : NPAD + 1, :], in_=zrow[:])

            # ---------------- super-steps
            for s in range(S):
                tab = h_t[0:HALF, :] if s < s_lo else h_t[HALF:N, :]
                ig = ldi.tile([128, 256], i16, tag="ig")
                nc.sync.dma_start(out=ig[:], in_=idxg_t[s])
                dr = ldi.tile([128, K], f32, tag="dr")
                nc.sync.dma_start(out=dr[:], in_=drel_t[s])
                aw_i = ldi.tile([128, 64], i16, tag="awi")
                nc.sync.dma_start(out=aw_i[:], in_=awi_t[s])
                sc_i = ldi.tile([128, 64], i16, tag="sci")
                nc.sync.dma_start(out=sc_i[:], in_=sci_t[s])

                Z = big.tile([128, K, F], f32, tag="Z")
                AW = med.tile([128, 8, F], f32, tag="AW")
                if STAGE < 1:
                    nc.vector.memset(Z[:], 0.01)
                    nc.vector.memset(AW[:], 0.01)
                if STAGE >= 1:
                    # SWDGE desc ring holds 128 descs/engine -> <=~2000
                    # idxs per instruction; use 4x1024 (65 descs each)
                    for q in range(4):
                        nc.gpsimd.dma_gather(
                            out_ap=Z[:, 8 * q : 8 * q + 8, :],
                            in_ap=tab,
                            idxs_ap=ig[:, 64 * q : 64 * q + 64],
                            num_idxs=1024, num_idxs_reg=1024, elem_size=F)
                if STAGE >= 2:
                    nc.gpsimd.dma_gather(out_ap=AW[:], in_ap=ta_t[:],
                                         idxs_ap=aw_i[:], num_idxs=1024,
                                         num_idxs_reg=1024, elem_size=F)
                elif STAGE == 1:
                    nc.vector.memset(AW[:], 0.01)

                # one-hot S[e, c, w] = (dst_rel[e,c] == w)
                St = big.tile([128, K, WMAX], f32, tag="St")
                nc.vector.tensor_tensor(
                    out=St[:],
                    in0=dr[:, :, None].to_broadcast([128, K, WMAX]),
                    in1=iota32[:, None, :].to_broadcast([128, K, WMAX]),
                    op=OP.is_equal)

                # S^T via PE transposes (4 chunks per 128-col block)
                sT = big.tile([128, 8, 128], f32, tag="sT")
                if STAGE >= 3:
                    st_ps = ps_st.tile([128, 8, 128], f32, tag="stp")
                    for b in range(8):
                        nc.tensor.transpose(
                            out=st_ps[:, b, :],
                            in_=St[:, 4 * b : 4 * b + 4, :].rearrange(
                                "p a b -> p (a b)"),
                            identity=ident[:])
                    nc.scalar.copy(out=sT[:], in_=st_ps[:])
                else:
                    nc.vector.memset(sT[:], 0.0)

                # d = a_dst[dst[e]] per slot, 4 chunks per matmul via jmask
                dps = ps_d.tile([128, K], f32, tag="dps")
                if STAGE >= 4:
                    for g in range(8):
                        aw4 = tiny.tile([128, 4], f32, tag="aw4")
                        nc.vector.tensor_tensor(out=aw4[:], in0=AW[:, g, 0:4],
                                                in1=jmask[:], op=OP.mult)
                        nc.tensor.matmul(out=dps[:, 4 * g : 4 * g + 4],
                                         lhsT=sT[:, g, :], rhs=aw4[:],
                                         start=True, stop=True)
                else:
                    nc.vector.memset(dps[:], 0.0)

                # rhs = [Zw | 1]; s = rowsum(Zw)
                rhsT = big.tile([128, K, F + 1], f32, tag="rhsT")
                nc.vector.memset(rhsT[:, :, F : F + 1], 1.0)
                nc.vector.tensor_tensor(
                    out=rhsT[:, :, 0:F], in0=Z[:],
                    in1=w1t[:, None, :].to_broadcast([128, K, F]), op=OP.mult)
                sC = med.tile([128, K], f32, tag="sC")
                nc.vector.tensor_reduce(out=sC[:], in_=rhsT[:, :, 0:F],
                                        axis=mybir.AxisListType.X, op=OP.add)
                sd = med.tile([128, K], f32, tag="sd")
                nc.vector.tensor_tensor(out=sd[:], in0=sC[:], in1=dps[:],
                                        op=OP.add)
                # leaky relu: max(x, 0.01x), then p = exp(.)
                sd2 = med.tile([128, K], f32, tag="sd2")
                nc.vector.tensor_scalar_mul(sd2[:], sd[:], NEG_SLOPE)
                eT = med.tile([128, K], f32, tag="eT")
                nc.vector.tensor_tensor(out=eT[:], in0=sd[:], in1=sd2[:],
                                        op=OP.max)
                pT = med.tile([128, K], f32, tag="pT")
                nc.scalar.activation(out=pT[:], in_=eT[:], func=AF.Exp)

                # Sp = S * p  (in place)
                nc.vector.tensor_tensor(
                    out=St[:], in0=St[:],
                    in1=pT[:, :, None].to_broadcast([128, K, WMAX]),
                    op=OP.mult)

                # per-chunk one-hot matmul -> [window, pZw | r]
                orow = orows.tile([32, K, F + 1], f32, tag="orow")
                if STAGE >= 5:
                    for g in range(8):
                        po = ps_o.tile([32, 4, F + 1], f32, tag="po")
                        for j in range(4):
                            c = 4 * g + j
                            nc.tensor.matmul(out=po[0:32, j, :],
                                             lhsT=St[:, c, :], rhs=rhsT[:, c, :],
                                             start=True, stop=True)
                        nc.scalar.copy(out=orow[0:32, 4 * g : 4 * g + 4, :],
                                       in_=po[:])
                else:
                    nc.vector.memset(orow[:], 0.0)

                # relayout [32, (g,j), 65] -> [128 (=32j+w), 8 (=g), 65]
                # (scatter consumes src strictly as [128, nidx/128, elem])
                orow8 = orows.tile([128, 8, F + 1], f32, tag="orow8")
                if STAGE >= 6:
                    for j in range(4):
                        nc.sync.dma_start(
                            out=orow8[32 * j : 32 * j + 32, :, :],
                            in_=orow[:, j :: 4, :])
                else:
                    nc.vector.memset(orow8[:], 0.0)

                if STAGE >= 7:
                    nc.gpsimd.dma_scatter_add(
                        out_ap=acc_t[:, 0 : F + 1], in_ap=orow8[:], idxs_ap=sc_i[:],
                        num_idxs=1024, num_idxs_reg=1024, elem_size=F + 1,
                        elem_step=128)

            # ---------------- final: divide by r and undo w1 fold
            accv = pre.tile([128, TCOLS, F + 1], f32)
            nc.sync.dma_start(
                out=accv[:],
                in_=acc_t[0:NPAD, 0 : F + 1].rearrange("(p t) c -> p t c",
                                                       p=128))
            rmax = pre.tile([128, TCOLS], f32)
            nc.vector.tensor_scalar_max(rmax[:], accv[:, :, F], 1e-30)
            rcp = pre.tile([128, TCOLS], f32)
            nc.vector.reciprocal(rcp[:], rmax[:])
            outv = pre.tile([128, TCOLS, F], f32)
            nc.vector.tensor_tensor(
                out=outv[:], in0=accv[:, :, 0:F],
                in1=rcp[:, :, None].to_broadcast([128, TCOLS, F]), op=OP.mult)
            nc.vector.tensor_tensor(
                out=outv[:], in0=outv[:],
                in1=rw1[:, None, :].to_broadcast([128, TCOLS, F]), op=OP.mult)
            nc.sync.dma_start(
                out=out_t[:].rearrange("(p t) f -> p t f", p=128),
                in_=outv[:])
    nc.compile()
    return nc


_prog_cache = {}


def kernel(h, attn_w, src, dst):
    from concourse.bass_utils import run_bass_kernel_spmd

    h = np.ascontiguousarray(h, dtype=np.float32)
    attn_w = np.ascontiguousarray(attn_w, dtype=np.float32)
    src = np.asarray(src, dtype=np.int32)
    dst = np.asarray(dst, dtype=np.int32)

    per_core = []
    for d in range(NCORES):
        n0 = d * NLOC
        e0, e1 = np.searchsorted(dst, [n0, n0 + NLOC])
        per_core.append(_prep_core(src[e0:e1], dst[e0:e1] - n0))

    s_lo = max(-(-len(st[0][2]) // K) for st in per_core)
    s_hi = max(-(-len(st[1][2]) // K) for st in per_core)

    key = (s_lo, s_hi)
    if key not in _prog_cache:
        _prog_cache[key] = _build_program(s_lo, s_hi)
    nc = _prog_cache[key]

    in_maps = []
    for d in range(NCORES):
        n0 = d * NLOC
        idxg, drel, awi, sci = _build_arrays(per_core[d], s_lo, s_hi)
        S = s_lo + s_hi
        h_slice = np.zeros((NPAD, F), np.float32)
        h_slice[:NLOC] = h[n0 : n0 + NLOC]
        in_maps.append({
            "h": h,
            "h_slice": h_slice,
            "attn_w": attn_w,
            "idxg": np.stack([_wrap16(idxg[s]) for s in range(S)]),
            "drel": drel,
            "awi": np.stack([_wrap16(awi[s].T.ravel()) for s in range(S)]),
            "sci": np.stack([_wrap16(sci[s]) for s in range(S)]),
        })

    global _last_in_maps
    _last_in_maps = in_maps
    res = run_bass_kernel_spmd(nc, in_maps, list(range(NCORES)))
    out = np.concatenate([res.results[d]["out"][:NLOC] for d in range(NCORES)])
    return out.astype(np.float32)


if __name__ == "__main__":
    import reference

    inputs = reference.setup_inputs()
    inputs = {k: np.asarray(v) for k, v in inputs.items()}
    got = kernel(**inputs)
    exp = np.asarray(reference.reference(**inputs))
    denom = np.abs(exp).max()
    rel = np.abs(got - exp).max() / denom
    print("Relative error:", rel)
